# revision 29
# baseline (speedup 1.0000x reference)
"""Trainium2 Bass kernel for the CIN-style layer:

    z   = einsum('btf,byf->bfty', x_0, x_k)            # pairwise outer products
    z   = z.reshape(bs, ts0, f, tsk)                   # flat reinterpretation
    out = einsum('btiy,nty->bni', z, conv_w) + conv_b  # strided conv reduction

Shapes: x_0 (32, 64, 256), x_k (32, 64, 256), conv_w (128, 64, 64),
conv_b (128,) -> out (32, 128, 256).

Math: with i = a*64 + m  (a = i//64, m = i%64) and feature f = 4t + a the
reference reduces to a two-step factorization:

    W2[b,a][t,n]     = sum_y x_k[b,y,4t+a] * conv_w[n,t,y]          (contract y)
    out[b,n,a*64+m]  = sum_t x_0[b,m,4t+a] * W2[b,a][t,n] + bias[n] (contract t)

Sharding (v18): 2D grid over (batch x t): core r handles batch group
g = r%4 (8 samples) and t-half H = r//4 (t in [32H, 32H+32)).  Each core
produces a PARTIAL output (its t-half's contribution; the bias is added
only into the H=0 partial); the host sums the two partials per batch
group.  This halves the replicated weight traffic (512 KB/core vs 1 MB).

Per-core indices: b in [0,8), a in [0,4), c = 4b+a in [0,32),
t' in [0,32) (global t = 32H+t'), pair p in [0,16), q in {0,1}
(t' = 2p+q), n in [0,128), y in [0,64), m in [0,64).
c-decomposition: j = b, (w, e) = (a//2, a%2), cp = 2b + a//2 = c//2.

  step 1 (n-stationary): 16 matmuls, one per t'-pair p:
      lhsT = WT[:, 128p:+128]     [K=128 (q,y), M=128 (n)]       (bf16)
      rhs  = XKpad[:, 64p:+64]    [K=128 (q,y), N=64 (b,a,q')]   (bf16,
             zero-padded block-diagonal in q==q', shipped padded)
      -> PSUM [n; 64(p%8) + 8b + 2a + q'] = W2[c][t'=2p+q', n]
  scatter (DVE, one per u=p//8): PSUM -> SBUF bf16
      w2n[n; 128b + 32a + t']   (t' = 16u + 2(p%8) + q')
  shuffle: 8 identity matmuls: lhsT = w2n[:, 128b:+128] -> PSUM
      T_b[32a + t'; n]; DVE casts -> w2r_v bf16 [128, 512] (v = b//4).
  step 2: 16 matmuls, one per cp = 2b + a//2  (K=128 = (w'', e, t')):
      lhsT = X0pad[:, 128cp:+128]  [128, 128 (e',m)]  (bf16, shipped
             zero-padded block-diagonal in BOTH w''==w and e'==e; the
             hardware rejects operands at partition offset 64, so the
             padding keeps every operand full-height at offset 0)
      rhs  = w2r[b//4][:, 128(b%4):+128]
      -> PSUM ps2[cp//4][64e'+m; 128(cp%4)+n]
  out: 4 quarter copies (DVE: q0,q2 / ACT: q1,q3) -> bf16, 4 DMAs
      (sync: q0,q2 / scalar: q1,q3).  The host upcasts, sums the two
      t-half partials per batch group, and adds conv_b in f32 (the bias
      never touches the device).

DMA: everything rides the sync ring in need order -- xk-dense, wt chunk
0, wt chunk 1 (identity rides its tail; the shuffle needs it late), x0pad
in two chunks (step-2 quarter 0 only waits the first) -- because
transfers drain in global issue order and each dma_start costs ~0.7us of
issue time on its engine.  The xk block-diagonal padding is rebuilt
on-chip (gpsimd memset early + two DVE copies as soon as xk lands, well
before the weights).  DMA completion semaphores tick +1 per SDMA engine
with a 0.3-1.2us straggler tail; consumers start ~50ns after the 16th
tick.  The shuffle casts are tiered (block 0 -> [128,128] w2r_a via
ps1[0], blocks 1-3 -> w2r_b, 4-7 -> w2r_c) so step 2 starts one small
cast after the first transpose instead of a full [128,512] cast later.
PSUM budget: ps1 x2 + ps_t x2 + ps2 x4 = all 8 banks (warm-up matmuls
reuse ps1[0], and ps1[0] is reused again for the block-0 transpose).
"""

import numpy as np
import ml_dtypes

BS, TS, F, NF = 32, 64, 256, 128
NCORES = 8
B = 8          # local batch per core (batch group)
NG = 4         # batch groups
TH = 32        # t-half length

F32 = np.float32
BF16 = ml_dtypes.bfloat16


# ---------------------------------------------------------------------------
# Host-side packing
# ---------------------------------------------------------------------------

def _pack_wt(conv_w: np.ndarray, H: int) -> np.ndarray:
    # WT[64q+y, 128p+n] = conv_w[n, 32H+2p+q, y]
    wt = conv_w[:, 32 * H:32 * (H + 1), :]               # [n, t', y]
    wt = wt.reshape(NF, 16, 2, 64)                       # [n, p, q, y]
    wt = wt.transpose(2, 3, 1, 0)                        # [q, y, p, n]
    return np.ascontiguousarray(wt.reshape(128, 2048).astype(BF16))


def _pack_xk_dense(xk_shard: np.ndarray, H: int) -> np.ndarray:
    # XKD[64q+y, 32p + 4b + a] = xk[b, y, 128H+8p+4q+a]  (dense; the
    # q==q' block-diagonal zero-padding is rebuilt on-chip)
    xs = xk_shard[:, :, 128 * H:128 * (H + 1)]           # [b, y, 8p+4q+a]
    xs = xs.reshape(B, TS, 16, 2, 4)                     # [b, y, p, q, a]
    dense = xs.transpose(3, 1, 2, 0, 4)                  # [q, y, p, b, a]
    return np.ascontiguousarray(dense.reshape(128, 512).astype(BF16))


def _pack_x0_padded(x0_shard: np.ndarray, H: int) -> np.ndarray:
    # X0pad[64w'' + 32e + t', 128cp + 64e' + m]
    #   = (w''==w) * (e'==e) * x0[b, m, 128H+4t'+a]
    # with cp = 2b + w, a = 2w + e.  Block-diagonal in BOTH w and e so that
    # step-2 matmuls read full K=128 operands at partition offset 0 (the
    # hardware rejects operands at partition offset 64).
    xs = x0_shard[:, :, 128 * H:128 * (H + 1)]           # [b, m, 4t'+a]
    xs = xs.reshape(B, TS, TH, 2, 2)                     # [b, m, t', w, e]
    src = xs.transpose(0, 3, 4, 2, 1)                    # [b, w, e, t', m]
    # [w'', e, t', b, w, e', m]
    pad = np.zeros((2, 2, TH, B, 2, 2, TS), dtype=F32)
    for w in range(2):
        for e in range(2):
            pad[w, e, :, :, w, e, :] = src[:, w, e].transpose(1, 0, 2)
    return np.ascontiguousarray(pad.reshape(128, 2048).astype(BF16))


def _unpack_out(out_pack: np.ndarray, out_full: np.ndarray, r: int) -> None:
    # out_pack[64e'+m, 128cp+n] = partial_out[b, n, a*64+m],
    # cp = 2b + a//2, e' = a%2; core r: batch group g=r%4, t-half H=r//4.
    # H=0 assigns, H=1 accumulates (callers iterate r=0..7 in order).
    g, H = r % 4, r // 4
    o = out_pack.astype(F32).reshape(2, TS, B, 2, NF)    # [e', m, b, w, n]
    part = o.transpose(2, 4, 3, 0, 1)                    # [b, n, w, e', m]
    part = np.ascontiguousarray(part).reshape(B, NF, F)  # a = 2w + e'
    if H == 0:
        out_full[B * g:B * (g + 1)] = part
    else:
        out_full[B * g:B * (g + 1)] += part


# ---------------------------------------------------------------------------
# Device program
# ---------------------------------------------------------------------------

_prog_cache = {}


def _emit_body(nc, tc, pool, ps_pool, f32, bf16, xkid_d, wt_d, x0_d,
               out_d, n_warm=2):
    # ---- input DMAs ----
    # xk rides the scalar ring so its issue overlaps the sync ring's wt0
    # issue: the two first transfers start draining ~0.7us earlier.  Both
    # drain-order outcomes of the issue race are fine: step-1 waits
    # max(xk_pad, wt0-sem) ~= 10.1us either way.
    xkid_s = pool.tile([128, 512], bf16, tag="xkid")
    nc.scalar.dma_start(xkid_s[:], xkid_d.ap())
    # tiny identity transfer first on sync: a deterministic spacer so xk
    # (scalar ring) wins the drain race against the weights; the identity
    # itself is only needed by the shuffle, much later.
    ident_s = pool.tile([128, 128], bf16, tag="ident")
    nc.sync.dma_start(ident_s[:], wt_d.ap()[:, 2048:2176])
    wt0_s = pool.tile([128, 1024], bf16, tag="wt0")
    nc.sync.dma_start(wt0_s[:], wt_d.ap()[:, 0:1024])
    wt1_s = pool.tile([128, 1024], bf16, tag="wt1")
    nc.sync.dma_start(wt1_s[:], wt_d.ap()[:, 1024:2048])
    wt_s = [wt0_s, wt1_s]
    x0p_a = pool.tile([128, 1024], bf16, tag="x0pa")
    nc.sync.dma_start(x0p_a[:], x0_d.ap()[:, 0:1024])
    x0p_b = pool.tile([128, 1024], bf16, tag="x0pb")
    nc.sync.dma_start(x0p_b[:], x0_d.ap()[:, 1024:2048])

    xk_dense = xkid_s[:, 0:512]
    ident = ident_s[:, :]

    # rebuild the q==q' block-diagonal xk padding on-chip: gpsimd zeroes
    # the tile early (no deps), two DVE copies land the dense halves as
    # soon as xk arrives (well before the weights).
    xk_pad = pool.tile([128, 1024], bf16, tag="xkpad")
    nc.gpsimd.memset(xk_pad[:], 0.0)
    for q in range(2):
        dst = xk_pad[64 * q:64 * (q + 1), :].rearrange(
            "z (pp c qp) -> z pp c qp", pp=16, c=32, qp=2)[:, :, :, q]
        src = xk_dense[64 * q:64 * (q + 1), :].rearrange(
            "z (pp c) -> z pp c", pp=16, c=32)
        nc.vector.tensor_copy(dst, src)

    # small constants (gpsimd, early, off the critical path)
    warm_s = pool.tile([128, 512], bf16, tag="warm")
    nc.gpsimd.memset(warm_s[:], 0.0)


    # PSUM: ps1 x2 (step 1), ps_t x2 (shuffle), ps2 x4 (step 2) = 8 banks
    ps1, ps_t, ps2 = [], [], []
    for i in range(2):
        ps1_i = ps_pool.tile([128, 512], f32, tag=f"s1_{i}")
        ps_t_i = ps_pool.tile([128, 512], f32, tag=f"t2_{i}")
        ps1.append(ps1_i)
        ps_t.append(ps_t_i)
    for i in range(4):
        ps2_i = ps_pool.tile([128, 512], f32, tag=f"s2_{i}")
        ps2.append(ps2_i)

    # PE warm-up (HAM clock ramp); reuses ps1[0], overwritten by step 1
    for _ in range(n_warm):
        nc.tensor.matmul(ps1[0][:, :], warm_s[:, 0:128], warm_s[:, :],
                         start=True, stop=True)

    # ---- step 1: W2 = wt_p.T @ xk_pad_p (contract (q,y)) ----
    w2n_s = pool.tile([128, 1024], bf16, tag="w2n")

    def emit_s1(u):
        for p in range(8 * u, 8 * u + 8):
            nc.tensor.matmul(
                ps1[u][:, 64 * (p % 8):64 * (p % 8 + 1)],
                wt_s[u][:, 128 * (p % 8):128 * (p % 8 + 1)],
                xk_pad[:, 64 * p:64 * (p + 1)],
                start=True, stop=True,
            )
        # scatter: psum cols (pp, b, a, q') -> w2n cols (b, a, u, pp, q')
        src = ps1[u][:, :].rearrange(
            "z (pp b a qp) -> z b a pp qp", pp=8, b=8, a=4)
        dst = w2n_s[:].rearrange(
            "z (b a uu pp qp) -> z b a uu pp qp",
            b=8, a=4, uu=2, pp=8, qp=2)[:, :, :, u, :, :]
        nc.vector.tensor_copy(dst, src)

    emit_s1(0)
    emit_s1(1)

    # ---- shuffle: T_b[32a+t'; n] via 8 identity matmuls; tiered DVE
    # casts so step 2 starts as soon as block 0 is transposed.  Block 0
    # lands in ps1[0] (free after scatter-u0), giving its cast a
    # single-matmul dependency; blocks 1-3 share ps_t[0], blocks 4-7
    # ps_t[1].
    w2r_a = pool.tile([128, 128], bf16, tag="w2ra")
    w2r_b = pool.tile([128, 384], bf16, tag="w2rb")
    w2r_c = pool.tile([128, 512], bf16, tag="w2rc")
    nc.tensor.matmul(ps1[0][:, 0:128], w2n_s[:, 0:128], ident,
                     start=True, stop=True)
    nc.vector.tensor_copy(w2r_a[:, :], ps1[0][:, 0:128])
    for b in range(1, 4):
        nc.tensor.matmul(
            ps_t[0][:, 128 * (b - 1):128 * b],
            w2n_s[:, 128 * b:128 * (b + 1)],
            ident,
            start=True, stop=True,
        )
    nc.vector.tensor_copy(w2r_b[:, :], ps_t[0][:, 0:384])
    for b in range(4, 8):
        nc.tensor.matmul(
            ps_t[1][:, 128 * (b % 4):128 * (b % 4 + 1)],
            w2n_s[:, 128 * b:128 * (b + 1)],
            ident,
            start=True, stop=True,
        )
    nc.vector.tensor_copy(w2r_c[:, :], ps_t[1][:, :])

    def w2r_cols(b):  # step-2 rhs tile [128, 128] for sample block b
        if b == 0:
            return w2r_a[:, 0:128]
        if b < 4:
            return w2r_b[:, 128 * (b - 1):128 * b]
        return w2r_c[:, 128 * (b % 4):128 * (b % 4 + 1)]

    # ---- step 2: ps2(bias) += x0pad.T @ w2r (contract (e,t'), K=64) ----
    out_q = []
    for qq in range(4):
        oq = pool.tile([128, 512], bf16, tag=f"out{qq}")
        out_q.append(oq)
    for cp in range(16):
        b, w = cp // 2, cp % 2
        qq, sidx = cp // 4, cp % 4
        x0t = x0p_a if cp < 8 else x0p_b
        nc.tensor.matmul(
            ps2[qq][:, 128 * sidx:128 * (sidx + 1)],
            x0t[:, 128 * (cp % 8):128 * (cp % 8 + 1)],
            w2r_cols(b),
            start=True, stop=True,
        )
        if cp % 4 == 3:
            if qq % 2 == 0:
                nc.vector.tensor_copy(out_q[qq][:, :], ps2[qq][:, :])
            else:
                nc.scalar.copy(out_q[qq][:, :], ps2[qq][:, :])
            eng = nc.sync if qq % 2 == 0 else nc.scalar
            eng.dma_start(out_d.ap()[:, 512 * qq:512 * (qq + 1)],
                          out_q[qq][:, :])


def _build_program(version=18):
    if version in _prog_cache:
        return _prog_cache[version]

    from contextlib import ExitStack

    import concourse.bacc as bacc
    import concourse.mybir as mybir
    import concourse.tile as tile

    f32 = mybir.dt.float32
    bf16 = mybir.dt.bfloat16
    nc = bacc.Bacc("TRN2", target_bir_lowering=False, debug=False)

    xkid_d = nc.dram_tensor("xkid_pack", [128, 512], bf16,
                            kind="ExternalInput")
    wt_d = nc.dram_tensor("wt_pack", [128, 2176], bf16, kind="ExternalInput")
    x0_d = nc.dram_tensor("x0_pack", [128, 2048], bf16, kind="ExternalInput")
    out_d = nc.dram_tensor("out_pack", [128, 2048], bf16,
                           kind="ExternalOutput")

    with tile.TileContext(nc) as tc, ExitStack() as ctx:
        pool = ctx.enter_context(tc.tile_pool(name="io", bufs=1))
        ps_pool = ctx.enter_context(tc.tile_pool(name="ps", bufs=1,
                                                 space="PSUM"))
        _emit_body(nc, tc, pool, ps_pool, f32, bf16, xkid_d, wt_d, x0_d,
                   out_d)

    nc.compile()
    _prog_cache[version] = nc
    return nc


def pack_core_inputs(x_0, x_k, conv_w, conv_b, version=18):
    """Returns in_maps (list of 8 dicts) for run_bass_kernel_spmd."""
    cw = np.asarray(conv_w, dtype=F32)
    ident = np.eye(128, dtype=BF16)
    wt_h = [np.ascontiguousarray(
        np.concatenate([_pack_wt(cw, H), ident], axis=1)) for H in range(2)]
    x0 = np.asarray(x_0, dtype=F32)
    xk = np.asarray(x_k, dtype=F32)
    in_maps = []
    for r in range(NCORES):
        g, H = r % 4, r // 4
        in_maps.append({
            "xkid_pack": _pack_xk_dense(xk[B * g:B * (g + 1)], H),
            "wt_pack": wt_h[H],
            "x0_pack": _pack_x0_padded(x0[B * g:B * (g + 1)], H),
        })
    return in_maps


VERSION = 18


def kernel(x_0, x_k, conv_w, conv_b):
    from concourse.bass_utils import run_bass_kernel_spmd

    nc = _build_program(VERSION)
    in_maps = pack_core_inputs(x_0, x_k, conv_w, conv_b, version=VERSION)
    res = run_bass_kernel_spmd(nc, in_maps, core_ids=list(range(NCORES)))
    out = np.empty((BS, NF, F), dtype=F32)
    for r in range(NCORES):
        _unpack_out(res.results[r]["out_pack"], out, r)
    out += np.asarray(conv_b, dtype=F32)[None, :, None]
    return out


# ---------------------------------------------------------------------------
# numpy model of the packed device program (for testing the packing logic)
# ---------------------------------------------------------------------------

def _numpy_model(x_0, x_k, conv_w, conv_b):
    out = np.empty((BS, NF, F), dtype=F32)
    in_maps = pack_core_inputs(x_0, x_k, conv_w, conv_b)
    for r in range(NCORES):
        m = in_maps[r]
        xkd = m["xkid_pack"].astype(F32)
        xk_pad = np.zeros((128, 1024), dtype=F32)
        for q in range(2):
            blk = xk_pad[64 * q:64 * (q + 1)].reshape(64, 16, 32, 2)
            blk[:, :, :, q] = xkd[64 * q:64 * (q + 1)].reshape(64, 16, 32)
        wt = m["wt_pack"][:, :2048].astype(F32)
        x0l = m["x0_pack"].astype(F32)
        # step 1 + scatter: w2n[n; 128b + 32a + t'], t' = 2p + q'
        w2n = np.zeros((128, 8, 4, TH), dtype=F32)       # [n, b, a, t']
        for p in range(16):
            blk = (wt[:, 128 * p:128 * (p + 1)].T
                   @ xk_pad[:, 64 * p:64 * (p + 1)])     # [n, (b,a,q')]
            blk = blk.reshape(128, 8, 4, 2)
            for qp in range(2):
                w2n[:, :, :, 2 * p + qp] = blk[:, :, :, qp]
        w2n = w2n.reshape(128, 1024).astype(BF16).astype(F32)
        # shuffle + cast
        w2r = np.zeros((128, 1024), dtype=F32)           # [(a,t'), 128b+n]
        for b in range(8):
            w2r[:, 128 * b:128 * (b + 1)] = w2n[:, 128 * b:128 * (b + 1)].T
        w2r = w2r.astype(BF16).astype(F32)
        # step 2 (psum pre-loaded with bias4)
        out_pack = np.zeros((128, 2048), dtype=F32)
        for cp in range(16):
            b = cp // 2
            out_pack[:, 128 * cp:128 * (cp + 1)] += (
                x0l[:, 128 * cp:128 * (cp + 1)].T
                @ w2r[:, 128 * b:128 * (b + 1)]
            )
        _unpack_out(out_pack.astype(BF16), out, r)
    out += np.asarray(conv_b, dtype=F32)[None, :, None]
    return out


# revision 30
# speedup vs baseline: 1.1086x; 1.1086x over previous
"""Trainium2 Bass kernel for the CIN-style layer:

    z   = einsum('btf,byf->bfty', x_0, x_k)            # pairwise outer products
    z   = z.reshape(bs, ts0, f, tsk)                   # flat reinterpretation
    out = einsum('btiy,nty->bni', z, conv_w) + conv_b  # strided conv reduction

Shapes: x_0 (32, 64, 256), x_k (32, 64, 256), conv_w (128, 64, 64),
conv_b (128,) -> out (32, 128, 256).

Math: with i = a*64 + m  (a = i//64, m = i%64) and feature f = 4t + a the
reference reduces to a two-step factorization:

    W2[b,a][t,n]     = sum_y x_k[b,y,4t+a] * conv_w[n,t,y]          (contract y)
    out[b,n,a*64+m]  = sum_t x_0[b,m,4t+a] * W2[b,a][t,n] + bias[n] (contract t)

Sharding (v18): 2D grid over (batch x t): core r handles batch group
g = r%4 (8 samples) and t-half H = r//4 (t in [32H, 32H+32)).  Each core
produces a PARTIAL output (its t-half's contribution; the bias is added
only into the H=0 partial); the host sums the two partials per batch
group.  This halves the replicated weight traffic (512 KB/core vs 1 MB).

Per-core indices: b in [0,8), a in [0,4), c = 4b+a in [0,32),
t' in [0,32) (global t = 32H+t'), pair p in [0,16), q in {0,1}
(t' = 2p+q), n in [0,128), y in [0,64), m in [0,64).
c-decomposition: j = b, (w, e) = (a//2, a%2), cp = 2b + a//2 = c//2.

  step 1 (n-stationary): 16 matmuls, one per t'-pair p:
      lhsT = WT[:, 128p:+128]     [K=128 (q,y), M=128 (n)]       (bf16)
      rhs  = XKpad[:, 64p:+64]    [K=128 (q,y), N=64 (b,a,q')]   (bf16,
             zero-padded block-diagonal in q==q', shipped padded)
      -> PSUM [n; 64(p%8) + 8b + 2a + q'] = W2[c][t'=2p+q', n]
  scatter (DVE, one per u=p//8): PSUM -> SBUF bf16
      w2n[n; 128b + 32a + t']   (t' = 16u + 2(p%8) + q')
  shuffle: 8 identity matmuls: lhsT = w2n[:, 128b:+128] -> PSUM
      T_b[32a + t'; n]; DVE casts -> w2r_v bf16 [128, 512] (v = b//4).
  step 2: 16 matmuls, one per cp = 2b + a//2  (K=128 = (w'', e, t')):
      lhsT = X0pad[:, 128cp:+128]  [128, 128 (e',m)]  (bf16, shipped
             zero-padded block-diagonal in BOTH w''==w and e'==e; the
             hardware rejects operands at partition offset 64, so the
             padding keeps every operand full-height at offset 0)
      rhs  = w2r[b//4][:, 128(b%4):+128]
      -> PSUM ps2[cp//4][64e'+m; 128(cp%4)+n]
  out: 4 quarter copies (DVE: q0,q2 / ACT: q1,q3) -> bf16, 4 DMAs
      (sync: q0,q2 / scalar: q1,q3).  The host upcasts, sums the two
      t-half partials per batch group, and adds conv_b in f32 (the bias
      never touches the device).

DMA: everything rides the sync ring in need order -- xk-dense, wt chunk
0, wt chunk 1 (identity rides its tail; the shuffle needs it late), x0pad
in two chunks (step-2 quarter 0 only waits the first) -- because
transfers drain in global issue order and each dma_start costs ~0.7us of
issue time on its engine.  The xk block-diagonal padding is rebuilt
on-chip (gpsimd memset early + two DVE copies as soon as xk lands, well
before the weights).  DMA completion semaphores tick +1 per SDMA engine
with a 0.3-1.2us straggler tail; consumers start ~50ns after the 16th
tick.  The shuffle casts are tiered (block 0 -> [128,128] w2r_a via
ps1[0], blocks 1-3 -> w2r_b, 4-7 -> w2r_c) so step 2 starts one small
cast after the first transpose instead of a full [128,512] cast later.
PSUM budget: ps1 x2 + ps_t x2 + ps2 x4 = all 8 banks (warm-up matmuls
reuse ps1[0], and ps1[0] is reused again for the block-0 transpose).
"""

import numpy as np
import ml_dtypes

BS, TS, F, NF = 32, 64, 256, 128
NCORES = 8
B = 8          # local batch per core (batch group)
NG = 4         # batch groups
TH = 32        # t-half length

F32 = np.float32
BF16 = ml_dtypes.bfloat16


# ---------------------------------------------------------------------------
# Host-side packing
# ---------------------------------------------------------------------------

def _pack_wt(conv_w: np.ndarray, H: int) -> np.ndarray:
    # WT[64q+y, 128p+n] = conv_w[n, 32H+2p+q, y]
    wt = conv_w[:, 32 * H:32 * (H + 1), :]               # [n, t', y]
    wt = wt.reshape(NF, 16, 2, 64)                       # [n, p, q, y]
    wt = wt.transpose(2, 3, 1, 0)                        # [q, y, p, n]
    return np.ascontiguousarray(wt.reshape(128, 2048).astype(BF16))


def _pack_xk_dense(xk_shard: np.ndarray, H: int) -> np.ndarray:
    # XKD[64q+y, 32p + 4b + a] = xk[b, y, 128H+8p+4q+a]  (dense; the
    # q==q' block-diagonal zero-padding is rebuilt on-chip)
    xs = xk_shard[:, :, 128 * H:128 * (H + 1)]           # [b, y, 8p+4q+a]
    xs = xs.reshape(B, TS, 16, 2, 4)                     # [b, y, p, q, a]
    dense = xs.transpose(3, 1, 2, 0, 4)                  # [q, y, p, b, a]
    return np.ascontiguousarray(dense.reshape(128, 512).astype(BF16))


def _pack_x0_padded(x0_shard: np.ndarray, H: int) -> np.ndarray:
    # X0pad[64w'' + 32e + t', 128cp + 64e' + m]
    #   = (w''==w) * (e'==e) * x0[b, m, 128H+4t'+a]
    # with cp = 2b + w, a = 2w + e.  Block-diagonal in BOTH w and e so that
    # step-2 matmuls read full K=128 operands at partition offset 0 (the
    # hardware rejects operands at partition offset 64).
    xs = x0_shard[:, :, 128 * H:128 * (H + 1)]           # [b, m, 4t'+a]
    xs = xs.reshape(B, TS, TH, 2, 2)                     # [b, m, t', w, e]
    src = xs.transpose(0, 3, 4, 2, 1)                    # [b, w, e, t', m]
    # [w'', e, t', b, w, e', m]
    pad = np.zeros((2, 2, TH, B, 2, 2, TS), dtype=F32)
    for w in range(2):
        for e in range(2):
            pad[w, e, :, :, w, e, :] = src[:, w, e].transpose(1, 0, 2)
    return np.ascontiguousarray(pad.reshape(128, 2048).astype(BF16))


def _unpack_out(out_pack: np.ndarray, out_full: np.ndarray, r: int) -> None:
    # out_pack[64e'+m, 128cp+n] = partial_out[b, n, a*64+m],
    # cp = 2b + a//2, e' = a%2; core r: batch group g=r%4, t-half H=r//4.
    # H=0 assigns, H=1 accumulates (callers iterate r=0..7 in order).
    g, H = r % 4, r // 4
    o = out_pack.astype(F32).reshape(2, TS, B, 2, NF)    # [e', m, b, w, n]
    part = o.transpose(2, 4, 3, 0, 1)                    # [b, n, w, e', m]
    part = np.ascontiguousarray(part).reshape(B, NF, F)  # a = 2w + e'
    if H == 0:
        out_full[B * g:B * (g + 1)] = part
    else:
        out_full[B * g:B * (g + 1)] += part


# ---------------------------------------------------------------------------
# Device program
# ---------------------------------------------------------------------------

_prog_cache = {}


def _emit_body(nc, tc, pool, ps_pool, f32, bf16, xkid_d, wt_d, x0_d,
               out_d, n_warm=2):
    # ---- input DMAs ----
    xkid_s = pool.tile([128, 512], bf16, tag="xkid")
    nc.sync.dma_start(xkid_s[:], xkid_d.ap())
    wt0_s = pool.tile([128, 1024], bf16, tag="wt0")
    nc.sync.dma_start(wt0_s[:], wt_d.ap()[:, 0:1024])
    wt1_s = pool.tile([128, 1152], bf16, tag="wt1")
    nc.sync.dma_start(wt1_s[:], wt_d.ap()[:, 1024:2176])
    wt_s = [wt0_s, wt1_s]
    x0p_a = pool.tile([128, 1024], bf16, tag="x0pa")
    nc.sync.dma_start(x0p_a[:], x0_d.ap()[:, 0:1024])
    x0p_b = pool.tile([128, 1024], bf16, tag="x0pb")
    nc.sync.dma_start(x0p_b[:], x0_d.ap()[:, 1024:2048])

    xk_dense = xkid_s[:, 0:512]
    ident = wt1_s[:, 1024:1152]

    # rebuild the q==q' block-diagonal xk padding on-chip: gpsimd zeroes
    # the tile early (no deps), two DVE copies land the dense halves as
    # soon as xk arrives (well before the weights).
    xk_pad = pool.tile([128, 1024], bf16, tag="xkpad")
    nc.gpsimd.memset(xk_pad[:], 0.0)
    for q in range(2):
        dst = xk_pad[64 * q:64 * (q + 1), :].rearrange(
            "z (pp c qp) -> z pp c qp", pp=16, c=32, qp=2)[:, :, :, q]
        src = xk_dense[64 * q:64 * (q + 1), :].rearrange(
            "z (pp c) -> z pp c", pp=16, c=32)
        nc.vector.tensor_copy(dst, src)

    # small constants (gpsimd, early, off the critical path)
    warm_s = pool.tile([128, 512], bf16, tag="warm")
    nc.gpsimd.memset(warm_s[:], 0.0)


    # PSUM: ps1 x2 (step 1), ps_t x2 (shuffle), ps2 x4 (step 2) = 8 banks
    ps1, ps_t, ps2 = [], [], []
    for i in range(2):
        ps1_i = ps_pool.tile([128, 512], f32, tag=f"s1_{i}")
        ps_t_i = ps_pool.tile([128, 512], f32, tag=f"t2_{i}")
        ps1.append(ps1_i)
        ps_t.append(ps_t_i)
    for i in range(4):
        ps2_i = ps_pool.tile([128, 512], f32, tag=f"s2_{i}")
        ps2.append(ps2_i)

    # PE warm-up (HAM clock ramp); reuses ps1[0], overwritten by step 1
    for _ in range(n_warm):
        nc.tensor.matmul(ps1[0][:, :], warm_s[:, 0:128], warm_s[:, :],
                         start=True, stop=True)

    # ---- step 1: W2 = wt_p.T @ xk_pad_p (contract (q,y)) ----
    w2n_s = pool.tile([128, 1024], bf16, tag="w2n")

    def emit_s1(u):
        for p in range(8 * u, 8 * u + 8):
            nc.tensor.matmul(
                ps1[u][:, 64 * (p % 8):64 * (p % 8 + 1)],
                wt_s[u][:, 128 * (p % 8):128 * (p % 8 + 1)],
                xk_pad[:, 64 * p:64 * (p + 1)],
                start=True, stop=True,
            )
        # scatter: psum cols (pp, b, a, q') -> w2n cols (b, a, u, pp, q')
        src = ps1[u][:, :].rearrange(
            "z (pp b a qp) -> z b a pp qp", pp=8, b=8, a=4)
        dst = w2n_s[:].rearrange(
            "z (b a uu pp qp) -> z b a uu pp qp",
            b=8, a=4, uu=2, pp=8, qp=2)[:, :, :, u, :, :]
        nc.vector.tensor_copy(dst, src)

    emit_s1(0)
    emit_s1(1)

    # ---- shuffle: T_b[32a+t'; n] via 8 identity matmuls; tiered DVE
    # casts so step 2 starts as soon as block 0 is transposed.  Block 0
    # lands in ps1[0] (free after scatter-u0), giving its cast a
    # single-matmul dependency; blocks 1-3 share ps_t[0], blocks 4-7
    # ps_t[1].
    w2r_a = pool.tile([128, 128], bf16, tag="w2ra")
    w2r_b = pool.tile([128, 384], bf16, tag="w2rb")
    w2r_c = pool.tile([128, 512], bf16, tag="w2rc")
    nc.tensor.matmul(ps1[0][:, 0:128], w2n_s[:, 0:128], ident,
                     start=True, stop=True)
    nc.vector.tensor_copy(w2r_a[:, :], ps1[0][:, 0:128])
    for b in range(1, 4):
        nc.tensor.matmul(
            ps_t[0][:, 128 * (b - 1):128 * b],
            w2n_s[:, 128 * b:128 * (b + 1)],
            ident,
            start=True, stop=True,
        )
    nc.vector.tensor_copy(w2r_b[:, :], ps_t[0][:, 0:384])
    for b in range(4, 8):
        nc.tensor.matmul(
            ps_t[1][:, 128 * (b % 4):128 * (b % 4 + 1)],
            w2n_s[:, 128 * b:128 * (b + 1)],
            ident,
            start=True, stop=True,
        )
    nc.vector.tensor_copy(w2r_c[:, :], ps_t[1][:, :])

    def w2r_cols(b):  # step-2 rhs tile [128, 128] for sample block b
        if b == 0:
            return w2r_a[:, 0:128]
        if b < 4:
            return w2r_b[:, 128 * (b - 1):128 * b]
        return w2r_c[:, 128 * (b % 4):128 * (b % 4 + 1)]

    # ---- step 2: ps2(bias) += x0pad.T @ w2r (contract (e,t'), K=64) ----
    out_q = []
    for qq in range(4):
        oq = pool.tile([128, 512], bf16, tag=f"out{qq}")
        out_q.append(oq)
    for cp in range(16):
        b, w = cp // 2, cp % 2
        qq, sidx = cp // 4, cp % 4
        x0t = x0p_a if cp < 8 else x0p_b
        nc.tensor.matmul(
            ps2[qq][:, 128 * sidx:128 * (sidx + 1)],
            x0t[:, 128 * (cp % 8):128 * (cp % 8 + 1)],
            w2r_cols(b),
            start=True, stop=True,
        )
        if cp % 4 == 3:
            if qq % 2 == 0:
                nc.vector.tensor_copy(out_q[qq][:, :], ps2[qq][:, :])
            else:
                nc.scalar.copy(out_q[qq][:, :], ps2[qq][:, :])
            eng = nc.sync if qq % 2 == 0 else nc.scalar
            eng.dma_start(out_d.ap()[:, 512 * qq:512 * (qq + 1)],
                          out_q[qq][:, :])


def _build_program(version=18):
    if version in _prog_cache:
        return _prog_cache[version]

    from contextlib import ExitStack

    import concourse.bacc as bacc
    import concourse.mybir as mybir
    import concourse.tile as tile

    f32 = mybir.dt.float32
    bf16 = mybir.dt.bfloat16
    nc = bacc.Bacc("TRN2", target_bir_lowering=False, debug=False)

    xkid_d = nc.dram_tensor("xkid_pack", [128, 512], bf16,
                            kind="ExternalInput")
    wt_d = nc.dram_tensor("wt_pack", [128, 2176], bf16, kind="ExternalInput")
    x0_d = nc.dram_tensor("x0_pack", [128, 2048], bf16, kind="ExternalInput")
    out_d = nc.dram_tensor("out_pack", [128, 2048], bf16,
                           kind="ExternalOutput")

    with tile.TileContext(nc) as tc, ExitStack() as ctx:
        pool = ctx.enter_context(tc.tile_pool(name="io", bufs=1))
        ps_pool = ctx.enter_context(tc.tile_pool(name="ps", bufs=1,
                                                 space="PSUM"))
        _emit_body(nc, tc, pool, ps_pool, f32, bf16, xkid_d, wt_d, x0_d,
                   out_d)

    nc.compile()
    _prog_cache[version] = nc
    return nc


def pack_core_inputs(x_0, x_k, conv_w, conv_b, version=18):
    """Returns in_maps (list of 8 dicts) for run_bass_kernel_spmd."""
    cw = np.asarray(conv_w, dtype=F32)
    ident = np.eye(128, dtype=BF16)
    wt_h = [np.ascontiguousarray(
        np.concatenate([_pack_wt(cw, H), ident], axis=1)) for H in range(2)]
    x0 = np.asarray(x_0, dtype=F32)
    xk = np.asarray(x_k, dtype=F32)
    in_maps = []
    for r in range(NCORES):
        g, H = r % 4, r // 4
        in_maps.append({
            "xkid_pack": _pack_xk_dense(xk[B * g:B * (g + 1)], H),
            "wt_pack": wt_h[H],
            "x0_pack": _pack_x0_padded(x0[B * g:B * (g + 1)], H),
        })
    return in_maps


VERSION = 18


def kernel(x_0, x_k, conv_w, conv_b):
    from concourse.bass_utils import run_bass_kernel_spmd

    nc = _build_program(VERSION)
    in_maps = pack_core_inputs(x_0, x_k, conv_w, conv_b, version=VERSION)
    res = run_bass_kernel_spmd(nc, in_maps, core_ids=list(range(NCORES)))
    out = np.empty((BS, NF, F), dtype=F32)
    for r in range(NCORES):
        _unpack_out(res.results[r]["out_pack"], out, r)
    out += np.asarray(conv_b, dtype=F32)[None, :, None]
    return out


# ---------------------------------------------------------------------------
# numpy model of the packed device program (for testing the packing logic)
# ---------------------------------------------------------------------------

def _numpy_model(x_0, x_k, conv_w, conv_b):
    out = np.empty((BS, NF, F), dtype=F32)
    in_maps = pack_core_inputs(x_0, x_k, conv_w, conv_b)
    for r in range(NCORES):
        m = in_maps[r]
        xkd = m["xkid_pack"].astype(F32)
        xk_pad = np.zeros((128, 1024), dtype=F32)
        for q in range(2):
            blk = xk_pad[64 * q:64 * (q + 1)].reshape(64, 16, 32, 2)
            blk[:, :, :, q] = xkd[64 * q:64 * (q + 1)].reshape(64, 16, 32)
        wt = m["wt_pack"][:, :2048].astype(F32)
        x0l = m["x0_pack"].astype(F32)
        # step 1 + scatter: w2n[n; 128b + 32a + t'], t' = 2p + q'
        w2n = np.zeros((128, 8, 4, TH), dtype=F32)       # [n, b, a, t']
        for p in range(16):
            blk = (wt[:, 128 * p:128 * (p + 1)].T
                   @ xk_pad[:, 64 * p:64 * (p + 1)])     # [n, (b,a,q')]
            blk = blk.reshape(128, 8, 4, 2)
            for qp in range(2):
                w2n[:, :, :, 2 * p + qp] = blk[:, :, :, qp]
        w2n = w2n.reshape(128, 1024).astype(BF16).astype(F32)
        # shuffle + cast
        w2r = np.zeros((128, 1024), dtype=F32)           # [(a,t'), 128b+n]
        for b in range(8):
            w2r[:, 128 * b:128 * (b + 1)] = w2n[:, 128 * b:128 * (b + 1)].T
        w2r = w2r.astype(BF16).astype(F32)
        # step 2 (psum pre-loaded with bias4)
        out_pack = np.zeros((128, 2048), dtype=F32)
        for cp in range(16):
            b = cp // 2
            out_pack[:, 128 * cp:128 * (cp + 1)] += (
                x0l[:, 128 * cp:128 * (cp + 1)].T
                @ w2r[:, 128 * b:128 * (b + 1)]
            )
        _unpack_out(out_pack.astype(BF16), out, r)
    out += np.asarray(conv_b, dtype=F32)[None, :, None]
    return out
